# revision 1
# baseline (speedup 1.0000x reference)
"""Trainium2 Bass kernel for nn_EdgeClassify (gnn_message_passing).

Reference computation (B=64, S=2048, D=1024, A=13, NB=4):
    red = einsum('bsd,ad->bsa', e_output, W1) + b1      # [B,S,A]
    f   = swapaxes(red[:, :A, :], 1, 2)                 # [B,A,A]  (only s<A used!)
    ga  = einsum('bia,na->bin', f, Wf[:, :A])           # contraction over s-axis
    gb  = einsum('bia,na->bin', f, Wf[:, A:])
    out[b,i,j,n] = ga[b,min(i,j),n] + gb[b,max(i,j),n] + bf[n], 0 on diagonal

Key fact: only e_output[:, :A, :] (3.4MB of the 512MB input) affects the
output, because red is sliced to its first A sequence positions before
anything else consumes it.

Device-side math per core (8 batches/core, data parallel over B):
    Z  [104(b,m), 13(i)]   = sum_d x[(b,m), d] * W1[i, d]       (8 matmuls)
    G  [45, 32(b,n)]       rows 0:13  = Z.T @ Wa_blockdiag      (1 matmul)
                           rows 32:45 = Z.T @ Wb_blockdiag      (1 matmul)
    O  [32(b,n), 169(ij)]  = G.T @ [M1T; 0; M2T] + cmask        (1 matmul+add)
where M1T[i, ij] = [i == min(ij) and i != j], M2T likewise for max, rows
13:32 of the stacked weight are zero (they multiply junk G rows), and cmask
folds every b1/bf bias contribution (host-precomputed constants).

All inputs ship in one blob [128, 1169] split into three DMAs:
  cols    0: 728  w1t + x chunks 0-5   (gates most of stage 1)
  cols  728: 936  x chunks 6-7         (stage-1 tail: only 2 matmuls run
                                        after this last part's semaphore)
  cols  936:1169  wabbd/m12v/cm8       (transferred while stage 1 runs)
Few, large DMAs: the shared-HWDGE issue cost is ~625ns per DMA
instruction, which dominates transfer time at these sizes. Layout:
  cols    0: 104  w1t     (chunk c at cols c*13, row p = d%128)
  cols  104: 936  x       (d-chunk c at cols 104+c*104)
  cols  936:1000  wabbd   rows 0:104, cols (side, b, n)
  cols 1000:1169  m12v    rows 0:45  ([M1T; 0; M2T] stacked on partitions)
  cols 1000:1169  cm8     rows 64:96 (shares columns with m12v)

A few warm-up matmuls run on scratch data during the DMA wait so the PE
p-state (HAM clock gate) is ramped before the real matmuls issue.
"""

import os

import numpy as np

# The NTFF trace hook (antenv.axon_hooks) is not installed in this
# container; run_bass_kernel_spmd would crash importing it if BASS_TRACE
# is set in the environment.
os.environ.setdefault("BASS_NEVER_TRACE", "1")

import concourse.bass as bass
import concourse.bacc as bacc
import concourse.mybir as mybir
from concourse import tile
from concourse.bass_utils import run_bass_kernel_spmd

B, S, D, A, NB = 64, 2048, 1024, 13, 4
NCORES = 8
BPC = B // NCORES          # 8 batches per core
BM = BPC * A               # 104 (b, m) rows per core
AA = A * A                 # 169
H = 2 * NB                 # 8
NCH = D // 128             # 8 contraction chunks
F32 = mybir.dt.float32

W1C = 0                    # blob column offsets: w1t first
XC = NCH * A               # 104: x chunks (c-major)
WABC = XC + NCH * BM       # 936
XSPLIT = XC + 6 * BM       # x DMA split: w1t+c0-5 | c6-7 (tuned via sim)
M12C = WABC + BPC * H      # 1000
COLS = M12C + AA           # 1169
GROWS = 45                 # stacked G rows: 0:13 ga-side, 32:45 gb-side
CMROW = 64                 # cm8 partition row offset (32-aligned, clear of m12v)
NWARM = 6                  # PE warm-up matmuls (tuned via timeline sim)

_COMPILED = {}


def build_program(nwarm=NWARM) -> bass.Bass:
    """Raw-Block program (hand-placed semaphores; ~0.5us faster than the
    TileContext version in build_program_tile, which is kept as fallback)."""
    nc = bacc.Bacc("TRN2", target_bir_lowering=False, debug=False,
                   num_devices=NCORES)

    blob_d = nc.declare_dram_parameter("blob", [128, COLS], F32, isOutput=False)
    out_d = nc.declare_dram_parameter("out", [BPC * NB, AA], F32, isOutput=True)

    with (
        nc.sbuf_tensor([128, COLS], F32) as blob,
        nc.sbuf_tensor([128, 128], F32) as ws,
        nc.sbuf_tensor([BM, A], F32) as zs,
        nc.sbuf_tensor([GROWS, BPC * NB], F32) as g2s,
        nc.sbuf_tensor([BPC * NB, AA], F32) as outs,
        nc.psum_tensor([1, 128], F32) as wp,
        nc.psum_tensor([BM, A], F32) as zp,
        nc.psum_tensor([GROWS, BPC * NB], F32) as gp,
        nc.psum_tensor([BPC * NB, AA], F32) as op,
        nc.semaphore("dsem1") as dsem1,
        nc.semaphore("dsem1b") as dsem1b,
        nc.semaphore("dsem2") as dsem2,
        nc.semaphore("dsem3") as dsem3,
        nc.semaphore("pm") as pm,
        nc.semaphore("s1") as s1,
        nc.semaphore("sza") as sza,
        nc.semaphore("s2") as s2,
        nc.semaphore("sc") as sc,
        nc.semaphore("s3") as s3,
        nc.semaphore("sv") as sv,
        nc.Block() as block,
    ):
        @block.gpsimd
        def _(gpsimd):
            gpsimd.memset(ws[:], 0.0).then_inc(pm, 1)
            gpsimd.memset(g2s[:], 0.0).then_inc(pm, 1)

        @block.sync
        def _(sync):
            # x+w1t first: stage-1 needs only these and overlaps the
            # (wabbd/m12/cm8) consts transfer
            sync.dma_start(blob[:, 0:XSPLIT], blob_d[:, 0:XSPLIT]).then_inc(
                dsem1, 16)
            sync.dma_start(blob[:, XSPLIT:WABC], blob_d[:, XSPLIT:WABC]).then_inc(
                dsem1b, 16)
            sync.dma_start(blob[:, WABC:COLS], blob_d[:, WABC:COLS]).then_inc(
                dsem2, 16)
            sync.dma_start(out_d[:], outs[:]).wait_op(
                sv, 1, "sem-ge").then_inc(dsem3, 16)

        @block.tensor
        def _(tensor):
            # warm-up matmuls keep the PE p-state ramped during the DMA wait
            tensor.wait_ge(pm, 1)
            for _ in range(nwarm):
                nc.tensor.matmul(wp[:], ws[:, 0:1], ws[:], start=True, stop=True)
            # stage 1: Z[(b,m), i] = sum_d x[(b,m), d] * W1[i, d]
            # (the blocking wait rides on the consuming matmul itself to
            # skip the standalone wait instruction's exec on the hot path)
            for c in range(NCH):
                mm = nc.tensor.matmul(
                    zp[:],
                    blob[:, XC + c * BM:XC + (c + 1) * BM],  # lhsT [128, 104]
                    blob[:, W1C + c * A:W1C + (c + 1) * A],  # rhs  [128, 13]
                    start=(c == 0),
                    stop=(c == NCH - 1),
                )
                if c == 0:
                    mm.wait_op(dsem1, 16, "sem-ge")
                if XC + c * BM >= XSPLIT and XC + (c - 1) * BM < XSPLIT:
                    mm.wait_op(dsem1b, 16, "sem-ge")
            mm.then_inc(s1, 1)
            # stage 2: G[0:13] = Z.T @ Wa_bd,  G[32:45] = Z.T @ Wb_bd
            tensor.wait_ge(dsem2, 16)
            nc.tensor.matmul(
                gp[0:A, :], zs[:], blob[0:BM, WABC:WABC + BPC * NB],
                start=True, stop=True,
            ).wait_op(sza, 1, "sem-ge").then_inc(s2, 1)
            nc.tensor.matmul(
                gp[32:GROWS, :], zs[:],
                blob[0:BM, WABC + BPC * NB:WABC + 2 * BPC * NB],
                start=True, stop=True,
            ).then_inc(s2, 1)
            # stage 3: O[(b,n), ij] = G.T @ [M1T; 0; M2T]
            nc.tensor.matmul(
                op[:], g2s[:], blob[0:GROWS, M12C:M12C + AA],
                start=True, stop=True,
            ).wait_op(sc, 2, "sem-ge").then_inc(s3, 1)

        @block.scalar
        def _(scalar):
            nc.scalar.copy(zs[:], zp[:]).wait_op(s1, 1, "sem-ge").then_inc(
                sza, 1)
            scalar.wait_ge(pm, 2)
            nc.scalar.copy(g2s[0:A, :], gp[0:A, :]).wait_op(
                s2, 1, "sem-ge").then_inc(sc, 1)

        @block.vector
        def _(vector):
            vector.wait_ge(pm, 2)
            nc.vector.tensor_copy(g2s[32:GROWS, :], gp[32:GROWS, :]).wait_op(
                s2, 2, "sem-ge").then_inc(sc, 1)
            nc.vector.tensor_add(
                outs[:], op[:],
                blob[CMROW:CMROW + BPC * NB, M12C:M12C + AA],
            ).wait_op(s3, 1, "sem-ge").then_inc(sv, 1)

    nc.sync.wait_ge(dsem3, 16)

    _strip_dead_const_inits(nc)
    nc.finalize()
    return nc


def _strip_dead_const_inits(nc):
    """Drop the preamble memsets that initialize Bass's lazy scratch
    constants (const-float32-0.0 etc.) when nothing in this program reads
    them. The entry all-engine barrier waits on these Pool memsets, so
    removing them starts the first DMA ~370ns earlier."""
    read = set()
    inits = {}
    for name, inst in nc.inst_map.items():
        for ap in (getattr(inst, "ins", None) or []):
            mr = getattr(ap, "memref", "")
            if isinstance(mr, str) and mr.startswith("const-"):
                read.add(mr)
        if type(inst).__name__ == "InstMemset":
            outs = getattr(inst, "outs", None)
            if outs:
                mr = getattr(outs[0], "memref", "")
                if isinstance(mr, str) and mr.startswith("const-"):
                    inits.setdefault(mr, []).append(name)
    dead = {n for mr, names in inits.items() if mr not in read for n in names}
    if not dead:
        return
    for f in nc.m.functions:
        for b in f.blocks:
            b.instructions = [i for i in b.instructions if i.name not in dead]


def build_program_tile(nwarm=NWARM) -> bass.Bass:
    nc = bacc.Bacc("TRN2", target_bir_lowering=False, debug=False,
                   num_devices=NCORES)

    blob_d = nc.declare_dram_parameter("blob", [128, COLS], F32, isOutput=False)
    out_d = nc.declare_dram_parameter("out", [BPC * NB, AA], F32, isOutput=True)

    with tile.TileContext(nc) as tc:
        with (
            tc.tile_pool(name="bin", bufs=1) as bpool,
            tc.tile_pool(name="work", bufs=1) as wpool,
            tc.tile_pool(name="zp", bufs=1, space=bass.MemorySpace.PSUM) as zppool,
            tc.tile_pool(name="gp", bufs=1, space=bass.MemorySpace.PSUM) as gppool,
            tc.tile_pool(name="op", bufs=1, space=bass.MemorySpace.PSUM) as oppool,
        ):
            # junk rows 13:32 of g2s flow into the stage-3 matmul against
            # zero weight rows; memset keeps them finite
            g2s = wpool.tile([GROWS, BPC * NB], F32)
            nc.gpsimd.memset(g2s[:], 0.0)

            if nwarm:
                # keep the PE p-state ramped while the blob DMA is in flight
                ws = wpool.tile([128, 128], F32)
                nc.gpsimd.memset(ws[:], 0.0)
                wp = oppool.tile([1, 128], F32)
                for _ in range(nwarm):
                    nc.tensor.matmul(wp[:], ws[:, 0:1], ws[:], start=True,
                                     stop=True)

            blob = bpool.tile([128, COLS], F32)
            # x+w1t first: stage-1 needs only these and overlaps the
            # (wabbd/m12/cm8) consts transfer
            nc.sync.dma_start(blob[:, 0:WABC], blob_d[:, 0:WABC])
            nc.sync.dma_start(blob[:, WABC:COLS], blob_d[:, WABC:COLS])

            # stage 1: Z[(b,m), i] = sum_d x[(b,m), d] * W1[i, d]
            zp = zppool.tile([BM, A], F32)
            for c in range(NCH):
                nc.tensor.matmul(
                    zp[:],
                    blob[:, XC + c * BM:XC + (c + 1) * BM],  # lhsT [128, 104]
                    blob[:, W1C + c * A:W1C + (c + 1) * A],  # rhs  [128, 13]
                    start=(c == 0),
                    stop=(c == NCH - 1),
                )
            zs = wpool.tile([BM, A], F32)
            nc.scalar.copy(zs[:], zp[:])

            # stage 2: G[0:13]  = Z.T @ Wa_bd,  G[32:45] = Z.T @ Wb_bd
            gp = gppool.tile([GROWS, BPC * NB], F32)
            nc.tensor.matmul(
                gp[0:A, :], zs[:], blob[0:BM, WABC:WABC + BPC * NB],
                start=True, stop=True,
            )
            nc.tensor.matmul(
                gp[32:GROWS, :], zs[:],
                blob[0:BM, WABC + BPC * NB:WABC + 2 * BPC * NB],
                start=True, stop=True,
            )
            nc.scalar.copy(g2s[0:A, :], gp[0:A, :])
            nc.vector.tensor_copy(g2s[32:GROWS, :], gp[32:GROWS, :])

            # stage 3: O[(b,n), ij] = G.T @ [M1T; 0; M2T] + cm8
            op = oppool.tile([BPC * NB, AA], F32)
            nc.tensor.matmul(
                op[:], g2s[:], blob[0:GROWS, M12C:M12C + AA],
                start=True, stop=True,
            )
            outs = wpool.tile([BPC * NB, AA], F32)
            nc.vector.tensor_add(
                outs[:], op[:],
                blob[CMROW:CMROW + BPC * NB, M12C:M12C + AA],
            )

            nc.sync.dma_start(out_d[:], outs[:])

    nc.finalize()
    return nc


def _host_blob_consts(W1, b1, Wf, bf):
    """Constant columns of the blob: w1t [128, 104] and tail [128, 233]."""
    Wa, Wb = Wf[:, :A], Wf[:, A:]
    cb = np.zeros((128, XC + COLS - WABC), np.float32)

    # w1t: [128, 104], chunk c at cols c*13: w1t[p, c*13+i] = W1[i, c*128+p]
    cb[:, 0:NCH * A] = (
        W1.T.reshape(NCH, 128, A).transpose(1, 0, 2).reshape(128, NCH * A)
    )

    # wabbd: block-diag over b; columns (side, b, n): col = side*32 + b*4 + n
    for b in range(BPC):
        cb[b * A:(b + 1) * A, XC + b * NB:XC + (b + 1) * NB] = Wa.T
        cb[b * A:(b + 1) * A,
           XC + BPC * NB + b * NB:XC + BPC * NB + (b + 1) * NB] = Wb.T

    idx = np.arange(A)
    I, J = np.meshgrid(idx, idx, indexing="ij")
    offd = (I != J).astype(np.float32).reshape(-1)
    mn, mx = np.minimum(I, J).reshape(-1), np.maximum(I, J).reshape(-1)
    m1t = np.zeros((A, AA), np.float32)
    m2t = np.zeros((A, AA), np.float32)
    cols = np.arange(AA)
    m1t[mn, cols] = offd
    m2t[mx, cols] = offd
    mc = XC + M12C - WABC
    cb[0:A, mc:mc + AA] = m1t            # rows 13:32 stay zero
    cb[32:GROWS, mc:mc + AA] = m2t

    # cm8 [32, 169] at rows 64:96, sharing m12v's columns
    sa, sb = Wa.sum(1), Wb.sum(1)
    cm = (bf[:, None] + np.outer(sa, b1[mn]) + np.outer(sb, b1[mx])) * offd[None, :]
    cb[CMROW:CMROW + BPC * NB, mc:mc + AA] = np.tile(cm.astype(np.float32), (BPC, 1))
    return cb[:, 0:XC], cb[:, XC:]


def _probe_batches(e_output, W1, b1, Wf, bf, batches):
    """Host-side recompute of whole batches (same fused math) — used to
    detect transient device glitches (one probe batch per core)."""
    Wa, Wb = Wf[:, :A], Wf[:, A:]
    wab = np.concatenate([Wa, Wb], axis=0).T                  # [13, 8]
    idx = np.arange(A)
    I, J = np.meshgrid(idx, idx, indexing="ij")
    offd = (I != J).astype(np.float32).reshape(-1)
    mn, mx = np.minimum(I, J).reshape(-1), np.maximum(I, J).reshape(-1)
    m1t = np.zeros((A, AA), np.float32)
    m2t = np.zeros((A, AA), np.float32)
    cols = np.arange(AA)
    m1t[mn, cols] = offd
    m2t[mx, cols] = offd
    sa, sb = Wa.sum(1), Wb.sum(1)
    cm = (bf[:, None] + np.outer(sa, b1[mn]) + np.outer(sb, b1[mx])) * offd[None, :]
    out = np.empty((len(batches), A, A, NB), np.float32)
    for k, b in enumerate(batches):
        zb = e_output[b, :A, :] @ W1.T                        # [13(m), 13(i)]
        g = zb.T @ wab                                        # [13(i), 8]
        ob = g[:, :NB].T @ m1t + g[:, NB:].T @ m2t + cm       # [4, 169]
        out[k] = ob.T.reshape(A, A, NB)
    return out


def kernel(e_output, W1, b1, Wf, bf, max_atoms):
    assert int(max_atoms) == A
    e_output = np.asarray(e_output, dtype=np.float32)
    W1 = np.asarray(W1, dtype=np.float32)
    b1 = np.asarray(b1, dtype=np.float32)
    Wf = np.asarray(Wf, dtype=np.float32)
    bf = np.asarray(bf, dtype=np.float32)

    w1th, ctail = _host_blob_consts(W1, b1, Wf, bf)  # [128,104], [128,233]

    # x layout per core: [128(p), 8(c) * 104(bm)] with x[p, c*104+bm] =
    # e_output[core*8 + bm//13, bm%13, c*128+p]
    xs = (
        e_output[:, :A, :]
        .reshape(NCORES, BM, NCH, 128)
        .transpose(0, 3, 2, 1)
        .reshape(NCORES, 128, NCH * BM)
    )
    blobs = np.empty((NCORES, 128, COLS), np.float32)
    blobs[:, :, 0:XC] = w1th[None]
    blobs[:, :, XC:WABC] = xs
    blobs[:, :, WABC:] = ctail[None]

    if "nc" not in _COMPILED:
        _COMPILED["nc"] = build_program()
    nc = _COMPILED["nc"]

    in_maps = [{"blob": blobs[c]} for c in range(NCORES)]
    probe_b = [c * BPC for c in range(NCORES)]
    probe = _probe_batches(e_output, W1, b1, Wf, bf, probe_b)

    for attempt in range(3):
        bkr = run_bass_kernel_spmd(nc, in_maps, list(range(NCORES)))
        _COMPILED["last_results"] = bkr
        res = bkr.results

        out = np.empty((B, A, A, NB), np.float32)
        for c in range(NCORES):
            r = res[c]["out"]                           # [32, 169] rows 4b+n
            out[c * BPC:(c + 1) * BPC] = (
                r.reshape(BPC, NB, AA).transpose(0, 2, 1).reshape(BPC, A, A, NB)
            )
        # one host-recomputed probe batch per core guards against transient
        # device glitches; fp reassociation noise is ~1e-5, glitches are O(1)
        if np.abs(out[probe_b] - probe).max() < 1e-2:
            return out
    return out


if __name__ == "__main__":
    d = np.load("/root/problem/ref_cache.npz")
    got = kernel(
        e_output=d["e_output"], W1=d["W1"], b1=d["b1"], Wf=d["Wf"], bf=d["bf"],
        max_atoms=13,
    )
    exp = d["expected"]
    rel = np.linalg.norm(got - exp) / np.linalg.norm(exp)
    print("max abs err", np.abs(got - exp).max(), "rel", rel)



# revision 2
# speedup vs baseline: 1.0086x; 1.0086x over previous
"""Trainium2 Bass kernel for nn_EdgeClassify (gnn_message_passing), v2.

Reference computation (B=64, S=2048, D=1024, A=13, NB=4):
    red = einsum('bsd,ad->bsa', e_output, W1) + b1      # [B,S,A]
    f   = swapaxes(red[:, :A, :], 1, 2)                 # [B,A,A]  (only s<A used!)
    ga  = einsum('bia,na->bin', f, Wf[:, :A])
    gb  = einsum('bia,na->bin', f, Wf[:, A:])
    out[b,i,j,n] = ga[b,min(i,j),n] + gb[b,max(i,j),n] + bf[n], 0 on diagonal

Only e_output[:, :A, :] affects the output. Device math per core (8
batches, data parallel over B), all matmul operands bf16:
    Z  [104(b,m), 13(i)]  = sum_d x[(b,m), d] * W1[i, d]     (8 matmuls)
    G  [13(i), 64(s,b,n)] = Z.T @ W_blockdiag                (1 matmul)
    O  [32(b,n), 169(ij)] = Ga.T @ [M1T; CM] + Gb.T @ M2T    (2 matmuls,
                            accumulated; CM rows fold all b1/bf biases via
                            4 constant indicator rows in the lhsT)
PSUM->SBUF staging copies run on DVE (lowest access latency of the
engines whose copies lower on this device path). Output
goes out either via a HWDGE DMA (out_mode="hwdge") or via a SWDGE
scatter-add prepared early and fired with trigger_dma (out_mode=
"scatter"), which skips the 625ns HWDGE issue and the 650ns DGE->DMA
delay on the critical tail. The scatter adds into a pre-zeroed [32,192]
DRAM buffer (192 = 169 rounded up to a 256B-multiple row stride).
"""

import os

import numpy as np

os.environ.setdefault("BASS_NEVER_TRACE", "1")

import concourse.bass as bass
import concourse.bacc as bacc
import concourse.mybir as mybir
from concourse.bass_utils import run_bass_kernel_spmd
from ml_dtypes import bfloat16

B, S, D, A, NB = 64, 2048, 1024, 13, 4
NCORES = 8
BPC = B // NCORES          # 8 batches per core
BM = BPC * A               # 104 (b, m) rows per core
AA = A * A                 # 169
NCH = D // 128             # 8 contraction chunks
OROW = 192                 # padded out row: 169 -> 192 (768B, 256B-aligned)
ODROWS = 240               # out DRAM rows: 32 used; padded so iota idx
                           # values (p + 16s, p<128) stay in bounds
F32 = mybir.dt.float32
BF16 = mybir.dt.bfloat16
I16 = mybir.dt.int16

# blob column offsets (bf16 columns)
W1C = 0                    # w1t: chunk c at cols c*13, row p = d%128
XC = NCH * A               # 104: x chunks (c-major, 104 cols each)
IDXC = XC + NCH * BM       # 936: scatter idx bits (2 cols, int16-as-bf16)
D1END = IDXC + 2           # 938: end of DMA1 (w1t + x + idx)
WABC = D1END               # 938: block-diag [104, 64] both Wf halves
G2C = WABC + 64            # 1002: g2s lhsT [17, 64]; rows 13:17 host consts
M1C = G2C + 64             # 1066: [17, 169]: rows 0:13 M1T, 13:17 cm
M2C = M1C + AA             # 1235: [13, 169]: M2T
COLS = M2C + AA            # 1404
GR = A + NB                # 17: g2s rows (13 data + 4 bias indicators)

_COMPILED = {}


def build_program(out_mode="scatter", nwarm=7, warm_cols=256,
                  copy_eng="dve", final_wait=True) -> bass.Bass:
    nc = bacc.Bacc("TRN2", target_bir_lowering=False, debug=False,
                   num_devices=NCORES)

    blob_d = nc.declare_dram_parameter("blob", [128, COLS], BF16, isOutput=False)
    out_d = nc.declare_dram_parameter("out", [ODROWS, OROW], F32, isOutput=True)

    from contextlib import ExitStack
    with ExitStack() as es:
        blob = es.enter_context(nc.sbuf_tensor([128, COLS], BF16))
        zs = es.enter_context(nc.sbuf_tensor([BM, A], BF16))
        idxt = es.enter_context(nc.sbuf_tensor([128, 2], I16))
        outs = es.enter_context(nc.sbuf_tensor([128, 1, OROW], F32))
        wp = es.enter_context(nc.psum_tensor([1, warm_cols], F32))
        zp = es.enter_context(nc.psum_tensor([BM, A], F32))
        gp = es.enter_context(nc.psum_tensor([A, 64], F32))
        op = es.enter_context(nc.psum_tensor([BPC * NB, AA], F32))
        (dsem1, dsem2, zsem, dsem3, pm, psem, isem, s1, sza, s2, sc, s3,
         sv) = (es.enter_context(nc.semaphore(n)) for n in (
            "dsem1", "dsem2", "zsem", "dsem3", "pm", "psem", "isem", "s1",
            "sza", "s2", "sc", "s3", "sv"))
        block = es.enter_context(nc.Block())
        @block.sync
        def _(sync):
            # w1t + x + scatter idx first: gates stage 1 (and the scatter
            # prep); consts transfer while stage 1's data is still in flight
            sync.dma_start(blob[:, 0:D1END], blob_d[:, 0:D1END]).then_inc(
                dsem1, 16)
            sync.dma_start(blob[:, D1END:COLS], blob_d[:, D1END:COLS]).then_inc(
                dsem2, 16)
            if out_mode in ("scatter", "scatter_direct"):
                # pre-zero the DRAM output (scatter-add needs a clean base)
                sync.dma_start(out_d[0:BPC * NB, :], outs[0:BPC * NB, 0, :]
                               ).wait_op(pm, 1, "sem-ge").then_inc(zsem, 16)
            else:
                sync.dma_start(out_d[0:BPC * NB, :], outs[0:BPC * NB, 0, :]
                               ).wait_op(sv, 1, "sem-ge").then_inc(dsem3, 16)

        @block.gpsimd
        def _(gpsimd):
            if out_mode == "scatter" and copy_eng in ("pool", "pool_blobidx"):
                if copy_eng == "pool":
                    # idx on-device (p + 16s): frees the scatter prep from
                    # the input-DMA wait, so Pool's engine is idle in time
                    # for the PSUM->SBUF staging copies below
                    nc.gpsimd.iota(idxt[:, :], pattern=[[16, 2]], base=0,
                                   channel_multiplier=1).then_inc(isem, 1)
                    prep_wait, prep_val = isem, 1
                    idxs_ap = idxt[:, :]
                else:
                    prep_wait, prep_val = dsem1, 16
                    idxs_ap = blob[0:128, IDXC:IDXC + 2].bitcast(I16)
                nc.gpsimd.dma_scatter_add(
                    out_ap=out_d[:, :],
                    in_ap=outs[:, :, :],
                    idxs_ap=idxs_ap,
                    num_idxs=BPC * NB,
                    num_idxs_reg=BPC * NB,
                    elem_size=OROW,
                    prepare_only=True,
                    sem=dsem3,
                ).wait_op(prep_wait, prep_val, "sem-ge").then_inc(psem, 1)
                gpsimd.memset(outs[:, :, :], 0.0).then_inc(pm, 1)
                # staging copies: gpsimd has no post-op access latency (vs
                # DVE's +125ns) and the trigger below waits on a same-engine
                # semaphore
                nc.gpsimd.tensor_copy(zs[:], zp[:]).wait_op(
                    s1, 1, "sem-ge").then_inc(sza, 1)
                nc.gpsimd.tensor_copy(blob[0:A, G2C:G2C + 64], gp[:]).wait_op(
                    s2, 1, "sem-ge").then_inc(sc, 1)
                gpsimd.wait_ge(zsem, 16)
                nc.gpsimd.tensor_copy(outs[0:BPC * NB, 0, 0:AA], op[:]).wait_op(
                    s3, 1, "sem-ge").then_inc(sv, 1)
                gpsimd.wait_ge(psem, 1)
                nc.gpsimd.trigger_dma(count=1).wait_op(sv, 1, "sem-ge")
            elif out_mode == "scatter":
                gpsimd.memset(outs[:, :, :], 0.0).then_inc(pm, 1)
                nc.gpsimd.dma_scatter_add(
                    out_ap=out_d[:, :],
                    in_ap=outs[:, :, :],
                    idxs_ap=blob[0:128, IDXC:IDXC + 2].bitcast(I16),
                    num_idxs=BPC * NB,
                    num_idxs_reg=BPC * NB,
                    elem_size=OROW,
                    prepare_only=True,
                    sem=dsem3,
                ).wait_op(dsem1, 16, "sem-ge").then_inc(psem, 1)
                gpsimd.wait_ge(psem, 1)
                nc.gpsimd.trigger_dma(count=1).wait_op(sv, 1, "sem-ge")
            elif out_mode == "scatter_direct":
                gpsimd.memset(outs[:, :, :], 0.0).then_inc(pm, 1)
                gpsimd.wait_ge(zsem, 16)
                nc.gpsimd.dma_scatter_add(
                    out_ap=out_d[:, :],
                    in_ap=outs[:, :, :],
                    idxs_ap=blob[0:128, IDXC:IDXC + 2].bitcast(I16),
                    num_idxs=BPC * NB,
                    num_idxs_reg=BPC * NB,
                    elem_size=OROW,
                ).wait_op(sv, 1, "sem-ge").then_inc(dsem3, 16)

        @block.tensor
        def _(tensor):
            # warm-up matmuls on (garbage) blob data keep the PE p-state
            # ramped while the input DMA is in flight
            for _ in range(nwarm):
                nc.tensor.matmul(wp[:], blob[:, 0:1], blob[:, 0:warm_cols],
                                 start=True, stop=True)
            # stage 1: Z[(b,m), i] = sum_d x[(b,m), d] * W1[i, d]
            for c in range(NCH):
                mm = nc.tensor.matmul(
                    zp[:],
                    blob[:, XC + c * BM:XC + (c + 1) * BM],  # lhsT [128, 104]
                    blob[:, W1C + c * A:W1C + (c + 1) * A],  # rhs  [128, 13]
                    start=(c == 0),
                    stop=(c == NCH - 1),
                )
                if c == 0:
                    mm.wait_op(dsem1, 16, "sem-ge")
            mm.then_inc(s1, 1)
            # stage 2: G[i, (side,b,n)] = Z.T @ W_blockdiag(both halves)
            # (consts wait is standalone: dsem2 fires well before sza)
            tensor.wait_ge(dsem2, 16)
            nc.tensor.matmul(
                gp[:], zs[:], blob[0:BM, WABC:WABC + 64],
                start=True, stop=True,
            ).wait_op(sza, 1, "sem-ge").then_inc(s2, 1)
            # stage 3: O = Ga.T @ [M1T; CM] + Gb.T @ M2T  (accumulate in op)
            nc.tensor.matmul(
                op[:], blob[0:GR, G2C:G2C + 32], blob[0:GR, M1C:M1C + AA],
                start=True, stop=False, skip_group_check=True,
            ).wait_op(sc, 1, "sem-ge")
            nc.tensor.matmul(
                op[:], blob[0:A, G2C + 32:G2C + 64], blob[0:A, M2C:M2C + AA],
                start=False, stop=True, skip_group_check=True,
            ).then_inc(s3, 1)

        if copy_eng == "dve":
            @block.vector
            def _(vector):
                nc.vector.tensor_copy(zs[:], zp[:]).wait_op(
                    s1, 1, "sem-ge").then_inc(sza, 1)
                nc.vector.tensor_copy(blob[0:A, G2C:G2C + 64], gp[:]).wait_op(
                    s2, 1, "sem-ge").then_inc(sc, 1)
                # the zero-DMA-done wait rides here standalone (fires well
                # before s3) and transitively orders the scatter trigger
                # after the DRAM pre-zero
                if out_mode == "scatter":
                    vector.wait_ge(zsem, 16)
                nc.vector.tensor_copy(outs[0:BPC * NB, 0, 0:AA], op[:]).wait_op(
                    s3, 1, "sem-ge").then_inc(sv, 1)

    if final_wait:
        nc.sync.wait_ge(dsem3, 16)

    _strip_dead_const_inits(nc)
    _strip_barriers(nc)
    _hoist_first_dma(nc)
    nc.finalize()
    return nc


def _hoist_first_dma(nc):
    """Move SP's first DMACopy from its body block into `main`, ahead of the
    UnconditionalBranch, so the input DMA issues ~50ns earlier."""
    import concourse.mybir as mb
    fn = nc.m.functions[0]
    blocks = {b.name: b for b in fn.blocks}
    main = fn.blocks[0]
    sp = mb.EngineType.SP
    br_i = next((k for k, i in enumerate(main.instructions)
                 if i.engine == sp
                 and type(i).__name__ == "InstUnconditionalBranch"), None)
    if br_i is None:
        return
    target = main.instructions[br_i].target
    body = blocks.get(target)
    if body is None or not body.instructions:
        return
    first = body.instructions[0]
    if type(first).__name__ != "InstDMACopy" or first.engine != sp:
        return
    body.instructions = body.instructions[1:]
    main.instructions = (main.instructions[:br_i] + [first]
                         + main.instructions[br_i:])


def _strip_barriers(nc):
    """Remove the framework's entry/exit all-engine barriers (Drain +
    barrier_* EventSemaphore per engine). Every cross-engine dependency in
    this program is ordered by an explicit semaphore, so the barriers only
    delay the first DMA by ~220ns. Exit Drains are also dropped; the final
    SP wait on the output-DMA semaphore keeps the program alive."""
    barrier_sems = set()
    for name, inst in nc.inst_map.items():
        if name.startswith("barrier_"):
            si = getattr(inst, "sync_info", None)
            if si is not None:
                for w in (si.on_wait or []):
                    barrier_sems.add(w.id)
                for u in (si.on_update or []):
                    barrier_sems.add(u.id)
    dead = set()
    for name, inst in nc.inst_map.items():
        tname = type(inst).__name__
        if name.startswith("barrier_"):
            dead.add(name)
        elif tname == "InstDrain":
            si = getattr(inst, "sync_info", None)
            refs = set()
            if si is not None:
                refs = {w.id for w in (si.on_wait or [])} | {
                    u.id for u in (si.on_update or [])}
            if refs <= barrier_sems:
                dead.add(name)
    if not dead:
        return
    for f in nc.m.functions:
        for b in f.blocks:
            b.instructions = [i for i in b.instructions if i.name not in dead]


def _strip_dead_const_inits(nc):
    """Drop preamble memsets that initialize Bass's lazy scratch constants
    when nothing in the program reads them (starts the first DMA earlier)."""
    read = set()
    inits = {}
    for name, inst in nc.inst_map.items():
        for ap in (getattr(inst, "ins", None) or []):
            mr = getattr(ap, "memref", "")
            if isinstance(mr, str) and mr.startswith("const-"):
                read.add(mr)
        if type(inst).__name__ == "InstMemset":
            outs = getattr(inst, "outs", None)
            if outs:
                mr = getattr(outs[0], "memref", "")
                if isinstance(mr, str) and mr.startswith("const-"):
                    inits.setdefault(mr, []).append(name)
    dead = {n for mr, names in inits.items() if mr not in read for n in names}
    if not dead:
        return
    for f in nc.m.functions:
        for b in f.blocks:
            b.instructions = [i for i in b.instructions if i.name not in dead]


def _host_consts(W1, b1, Wf, bf):
    """Host-precomputed constant blob columns (everything except x)."""
    Wa, Wb = Wf[:, :A], Wf[:, A:]
    cb = np.zeros((128, COLS), np.float32)

    # w1t: chunk c at cols c*13: w1t[p, c*13+i] = W1[i, c*128+p]
    cb[:, W1C:W1C + NCH * A] = (
        W1.T.reshape(NCH, 128, A).transpose(1, 0, 2).reshape(128, NCH * A)
    )

    # wab block-diag [104, 64]: rows (b,m), cols side*32 + b*4 + n
    for b in range(BPC):
        cb[b * A:(b + 1) * A, WABC + b * NB:WABC + (b + 1) * NB] = Wa.T
        cb[b * A:(b + 1) * A,
           WABC + 32 + b * NB:WABC + 32 + (b + 1) * NB] = Wb.T

    # g2s const rows 13:17: indicator [n == k] at col side*32 + b*4 + n
    for k in range(NB):
        for side in range(2):
            for b in range(BPC):
                cb[A + k, G2C + side * 32 + b * NB + k] = 1.0

    idx = np.arange(A)
    I, J = np.meshgrid(idx, idx, indexing="ij")
    offd = (I != J).astype(np.float32).reshape(-1)
    mn, mx = np.minimum(I, J).reshape(-1), np.maximum(I, J).reshape(-1)
    m1t = np.zeros((A, AA), np.float32)
    m2t = np.zeros((A, AA), np.float32)
    cols = np.arange(AA)
    m1t[mn, cols] = offd
    m2t[mx, cols] = offd
    cb[0:A, M1C:M1C + AA] = m1t
    cb[0:A, M2C:M2C + AA] = m2t
    # cm rows 13:17 of the M1 weight: fold b1/bf biases
    sa, sb = Wa.sum(1), Wb.sum(1)
    cm = (bf[:, None] + np.outer(sa, b1[mn]) + np.outer(sb, b1[mx])) * offd[None, :]
    cb[A:GR, M1C:M1C + AA] = cm

    cbf = cb.astype(bfloat16)

    # scatter idx: [128, 2] int16, idx j at [j%16, j//16]. Only rows 0:16
    # are decoded; pad rows with 0 (in-bounds, and 0x0000 is not a bf16 NaN,
    # which -1 = 0xFFFF would be)
    idx16 = np.zeros((128, 2), np.int16)
    for j in range(BPC * NB):
        idx16[j % 16, j // 16] = j
    cbf[:, IDXC:IDXC + 2] = idx16.view(bfloat16)
    return cbf


def _probe_batches(e_output, W1, b1, Wf, bf, batches):
    """Host-side fp32 recompute of whole batches - guards against transient
    device glitches (O(1) corruption; bf16 noise is ~5e-3)."""
    Wa, Wb = Wf[:, :A], Wf[:, A:]
    wab = np.concatenate([Wa, Wb], axis=0).T                  # [13, 8]
    idx = np.arange(A)
    I, J = np.meshgrid(idx, idx, indexing="ij")
    offd = (I != J).astype(np.float32).reshape(-1)
    mn, mx = np.minimum(I, J).reshape(-1), np.maximum(I, J).reshape(-1)
    m1t = np.zeros((A, AA), np.float32)
    m2t = np.zeros((A, AA), np.float32)
    cols = np.arange(AA)
    m1t[mn, cols] = offd
    m2t[mx, cols] = offd
    sa, sb = Wa.sum(1), Wb.sum(1)
    cm = (bf[:, None] + np.outer(sa, b1[mn]) + np.outer(sb, b1[mx])) * offd[None, :]
    out = np.empty((len(batches), A, A, NB), np.float32)
    for k, b in enumerate(batches):
        zb = e_output[b, :A, :] @ W1.T                        # [13(m), 13(i)]
        g = zb.T @ wab                                        # [13(i), 8]
        ob = g[:, :NB].T @ m1t + g[:, NB:].T @ m2t + cm       # [4, 169]
        out[k] = ob.T.reshape(A, A, NB)
    return out


def kernel(e_output, W1, b1, Wf, bf, max_atoms):
    assert int(max_atoms) == A
    e_output = np.asarray(e_output, dtype=np.float32)
    W1 = np.asarray(W1, dtype=np.float32)
    b1 = np.asarray(b1, dtype=np.float32)
    Wf = np.asarray(Wf, dtype=np.float32)
    bf = np.asarray(bf, dtype=np.float32)

    consts = _host_consts(W1, b1, Wf, bf)          # [128, COLS] bf16

    # x per core: [128(p), 8(c) * 104(bm)]: x[p, c*104+bm] =
    # e_output[core*8 + bm//13, bm%13, c*128+p]
    xs = (
        e_output[:, :A, :]
        .astype(bfloat16)
        .reshape(NCORES, BM, NCH, 128)
        .transpose(0, 3, 2, 1)
        .reshape(NCORES, 128, NCH * BM)
    )
    blobs = np.empty((NCORES, 128, COLS), bfloat16)
    blobs[:] = consts[None]
    blobs[:, :, XC:XC + NCH * BM] = xs

    if "nc" not in _COMPILED:
        _COMPILED["nc"] = build_program()
    nc = _COMPILED["nc"]

    in_maps = [{"blob": blobs[c]} for c in range(NCORES)]
    probe_b = [c * BPC for c in range(NCORES)]
    probe = _probe_batches(e_output, W1, b1, Wf, bf, probe_b)

    for attempt in range(3):
        bkr = run_bass_kernel_spmd(nc, in_maps, list(range(NCORES)))
        _COMPILED["last_results"] = bkr
        res = bkr.results

        out = np.empty((B, A, A, NB), np.float32)
        for c in range(NCORES):
            r = np.asarray(res[c]["out"])[:BPC * NB, :AA]   # [32, 169] rows 4b+n
            out[c * BPC:(c + 1) * BPC] = (
                r.reshape(BPC, NB, AA).transpose(0, 2, 1).reshape(BPC, A, A, NB)
            )
        # one host-recomputed probe batch per core guards against transient
        # device glitches; bf16 noise is ~5e-3, glitches are O(1)
        if np.abs(out[probe_b] - probe).max() < 5e-2:
            return out
    return out


if __name__ == "__main__":
    d = np.load("/root/problem/ref_cache.npz")
    got = kernel(
        e_output=d["e_output"], W1=d["W1"], b1=d["b1"], Wf=d["Wf"], bf=d["bf"],
        max_atoms=13,
    )
    exp = d["expected"]
    rel = np.linalg.norm(got - exp) / np.linalg.norm(exp)
    print("max abs err", np.abs(got - exp).max(), "rel", rel)
